# revision 1
# baseline (speedup 1.0000x reference)
"""BitLinear (RMSNorm + per-row int8 activation quant + ternary GEMM + dequant)
on 8 Trainium2 NeuronCores.

Sharding: data-parallel over the 16384 (B*S) token rows -- 2048 rows per core,
w replicated. This minimizes HBM traffic (each core reads only its x shard plus
one copy of w) and avoids duplicating the RMSNorm/quant work.

Math notes:
  - Quantized activations are integers in [-127, 127] and weights are ternary
    {-1, 0, 1}: both exactly representable in bf16, so the GEMM runs on the
    TensorEngine in bf16 with f32 PSUM accumulation with zero rounding error
    (|acc| <= 127*4096 < 2^24).
  - round-half-to-even (jnp.round semantics) is implemented with the
    (v + 1.5*2^23) - 1.5*2^23 trick in f32 (IEEE RNE).
  - x is shipped twice (natural and transposed) so that the row statistics use
    free-dim reductions while the quantized K-major operand is produced without
    any on-chip transposes.
"""

import sys

if "/opt/trn_rl_repo" not in sys.path:
    sys.path.insert(0, "/opt/trn_rl_repo")

from contextlib import ExitStack

import ml_dtypes
import numpy as np

import concourse.bacc as bacc
import concourse.bass as bass
import concourse.mybir as mybir
import concourse.tile as tile
from concourse.bass import ts
from concourse.bass_utils import run_bass_kernel_spmd

F32 = mybir.dt.float32
BF16 = mybir.dt.bfloat16
AX = mybir.AxisListType
OP = mybir.AluOpType
ACTF = mybir.ActivationFunctionType

MAGIC = 12582912.0  # 1.5 * 2**23: (v + MAGIC) - MAGIC == round-to-nearest-even(v)
EPS = 1e-5
N_CORES = 8


def build_bitlinear(R, K, O, inv_sw127, rms_ones=True, o_blk=512, w_bufs_extra=16):
    """Single-core program. Inputs: x_nat [R,K] f32, x_t [K,R] f32,
    w_t [K,O] bf16 (pre-transposed [in,out]), optional rms [K] f32.
    Output: out [R,O] f32."""
    nbc = R // 128
    nkc = K // 128
    nob = O // o_blk
    assert R % 128 == 0 and K % 128 == 0 and O % o_blk == 0

    nc = bacc.Bacc("TRN2", target_bir_lowering=False, debug=False, num_devices=N_CORES)
    x_nat = nc.declare_dram_parameter("x_nat", [R, K], F32, isOutput=False)
    x_t = nc.declare_dram_parameter("x_t", [K, R], F32, isOutput=False)
    w_t = nc.declare_dram_parameter("w_t", [K, O], BF16, isOutput=False)
    rms = None
    if not rms_ones:
        rms = nc.declare_dram_parameter("rms", [K], F32, isOutput=False)
    out = nc.declare_dram_parameter("out", [R, O], F32, isOutput=True)

    with ExitStack() as ctx:
        tc = ctx.enter_context(tile.TileContext(nc))
        singles = ctx.enter_context(tc.tile_pool(name="singles", bufs=1))
        dpool = ctx.enter_context(tc.tile_pool(name="dpool", bufs=1, space="DRAM"))

        ssum = singles.tile([128, nbc], F32)  # per-row sum(x^2)
        mraw = singles.tile([128, nbc], F32)  # per-row max|x*w|
        dq = singles.tile([128, nbc], F32)  # per-row dequant scale
        s_dram = dpool.tile([nbc, 128], F32)  # bounce buffer: quant scale, bs-major

        w_rep = None
        rms_cols = None
        if not rms_ones:
            w_rep = singles.tile([128, K], F32)
            rms_bcast = bass.AP(
                tensor=rms.ap().tensor, offset=rms.ap().offset, ap=[[0, 128], [1, K]]
            )
            nc.sync.dma_start(out=w_rep, in_=rms_bcast)
            rms_cols = singles.tile([128, nkc], F32)
            for kk in range(nkc):
                nc.sync.dma_start(
                    out=rms_cols[:, kk : kk + 1], in_=rms.ap()[ts(kk, 128)]
                )

        # ---- stage 1: per-row stats (natural layout, free-dim reductions) ----
        with (
            tc.tile_pool(name="st1x", bufs=3) as st1x,
            tc.tile_pool(name="st1s", bufs=2) as st1s,
        ):
            for c in range(nbc):
                xt_ = st1x.tile([128, K], F32, tag="xt")
                nc.sync.dma_start(out=xt_, in_=x_nat[ts(c, 128), :])
                sq = st1s.tile([128, K], F32, tag="sq")
                nc.scalar.activation(
                    out=sq, in_=xt_, func=ACTF.Square, accum_out=ssum[:, c : c + 1]
                )
                if rms_ones:
                    nc.vector.tensor_reduce(
                        out=mraw[:, c : c + 1],
                        in_=xt_,
                        axis=AX.X,
                        op=OP.max,
                        apply_absolute_value=True,
                    )
                else:
                    p = st1s.tile([128, K], F32, tag="p")
                    nc.vector.tensor_mul(p, xt_, w_rep)
                    nc.vector.tensor_reduce(
                        out=mraw[:, c : c + 1],
                        in_=p,
                        axis=AX.X,
                        op=OP.max,
                        apply_absolute_value=True,
                    )

        # ---- stage 1b: batched per-row scalar math ([128, nbc] tiles) ----
        # a = ssum/K + eps; rstd = NR-refined 1/sqrt(a)
        a = singles.tile([128, nbc], F32)
        nc.vector.tensor_scalar(a, ssum, 1.0 / K, EPS, OP.mult, OP.add)
        ysq = singles.tile([128, nbc], F32)
        nc.scalar.activation(out=ysq, in_=a, func=ACTF.Sqrt)
        r0 = singles.tile([128, nbc], F32)
        nc.vector.reciprocal(r0, ysq)
        t1 = singles.tile([128, nbc], F32)
        nc.vector.tensor_mul(t1, r0, r0)
        t2 = singles.tile([128, nbc], F32)
        nc.vector.tensor_mul(t2, t1, a)
        t3 = singles.tile([128, nbc], F32)
        nc.vector.tensor_scalar(t3, t2, -0.5, 1.5, OP.mult, OP.add)
        rstd = singles.tile([128, nbc], F32)
        nc.vector.tensor_mul(rstd, r0, t3)
        # max_abs = max(mraw * rstd, 1e-5); dq = max_abs * (1/(127*scale_w))
        ma = singles.tile([128, nbc], F32)
        nc.vector.tensor_mul(ma, mraw, rstd)
        mac = singles.tile([128, nbc], F32)
        nc.vector.tensor_scalar(mac, ma, 1e-5, None, OP.max)
        nc.vector.tensor_scalar_mul(dq, mac, inv_sw127)
        # s = rstd * 127 / max_abs  (combined quantization scale per row)
        inv = singles.tile([128, nbc], F32)
        nc.vector.reciprocal(inv, mac)
        sc0 = singles.tile([128, nbc], F32)
        nc.vector.tensor_mul(sc0, inv, rstd)
        s_col = singles.tile([128, nbc], F32)
        nc.vector.tensor_scalar_mul(s_col, sc0, 127.0)
        # scatter-transpose s_col [128(bs_in_chunk), nbc] -> s_dram flat bs order
        s_dram_t = bass.AP(
            tensor=s_dram.tensor, offset=s_dram.offset, ap=[[1, 128], [128, nbc]]
        )
        nc.sync.dma_start(out=s_dram_t, in_=s_col)

        # broadcast-read back: s_rep[p, bs] = s[bs] for all 128 partitions
        s_rep = singles.tile([128, R], F32)
        s_bcast = bass.AP(
            tensor=s_dram.tensor, offset=s_dram.offset, ap=[[0, 128], [1, R]]
        )
        nc.sync.dma_start(out=s_rep, in_=s_bcast)

        # ---- stage 2: quantize in transposed layout -> xq (bf16, K-major) ----
        xqp = ctx.enter_context(tc.tile_pool(name="xqp", bufs=nkc))
        xq_tiles = []
        with (
            tc.tile_pool(name="st2x", bufs=3) as st2x,
            tc.tile_pool(name="st2t", bufs=2) as st2t,
        ):
            for kk in range(nkc):
                xtt = st2x.tile([128, R], F32, tag="xtt")
                nc.sync.dma_start(out=xtt, in_=x_t[ts(kk, 128), :])
                t = st2t.tile([128, R], F32, tag="t")
                nc.vector.tensor_mul(t, xtt, s_rep)
                xq = xqp.tile([128, R], BF16, tag="xq")
                if rms_ones:
                    nc.vector.tensor_scalar(xq, t, MAGIC, MAGIC, OP.add, OP.subtract)
                else:
                    t2_ = st2t.tile([128, R], F32, tag="t2_")
                    nc.vector.tensor_scalar(
                        t2_, t, rms_cols[:, kk : kk + 1], MAGIC, OP.mult, OP.add
                    )
                    nc.vector.tensor_scalar(xq, t2_, MAGIC, None, OP.subtract)
                xq_tiles.append(xq)

        # ---- stage 3: GEMM out[bs, o] = xq.T @ w_t, dequant, store ----
        wp = ctx.enter_context(tc.tile_pool(name="wp", bufs=nkc + w_bufs_extra))
        pp = ctx.enter_context(tc.tile_pool(name="pp", bufs=8, space="PSUM"))
        outp = ctx.enter_context(tc.tile_pool(name="outp", bufs=3))
        for ob in range(nob):
            wts = []
            for kk in range(nkc):
                wt_ = wp.tile([128, o_blk], BF16, tag="wt")
                nc.sync.dma_start(out=wt_, in_=w_t[ts(kk, 128), ts(ob, o_blk)])
                wts.append(wt_)
            for c in range(nbc):
                ps = pp.tile([128, o_blk], F32, tag="ps")
                for kk in range(nkc):
                    nc.tensor.matmul(
                        ps,
                        xq_tiles[kk][:, ts(c, 128)],
                        wts[kk],
                        start=(kk == 0),
                        stop=(kk == nkc - 1),
                    )
                ot = outp.tile([128, o_blk], F32, tag="ot")
                nc.scalar.activation(
                    out=ot, in_=ps, func=ACTF.Copy, scale=dq[:, c : c + 1]
                )
                nc.sync.dma_start(out=out[ts(c, 128), ts(ob, o_blk)], in_=ot)

    nc.compile()
    return nc


_NC_CACHE = {}


def _get_nc(R, K, O, inv_sw127, rms_ones):
    key = (R, K, O, float(inv_sw127), rms_ones)
    if key not in _NC_CACHE:
        _NC_CACHE[key] = build_bitlinear(R, K, O, inv_sw127, rms_ones=rms_ones)
    return _NC_CACHE[key]


def make_in_maps(x, rms_weight, w_ternary, scale_w, n_cores=N_CORES):
    """Host-side sharding/layout prep. Returns (in_maps, meta)."""
    x = np.asarray(x, dtype=np.float32)
    rms_weight = np.asarray(rms_weight, dtype=np.float32)
    w_ternary = np.asarray(w_ternary, dtype=np.float32)
    scale_w = np.asarray(scale_w, dtype=np.float32)

    B, S, K = x.shape
    Ofeat = w_ternary.shape[0]
    M = B * S
    assert M % n_cores == 0
    R = M // n_cores

    rms_ones = bool(np.all(rms_weight == np.float32(1.0)))
    sw = np.float32(scale_w.reshape(-1)[0])
    inv_sw127 = float(np.float32(1.0) / (np.float32(127.0) * sw))

    xf = x.reshape(M, K)
    w_t_bf = np.ascontiguousarray(w_ternary.T).astype(ml_dtypes.bfloat16)

    in_maps = []
    for i in range(n_cores):
        xs = np.ascontiguousarray(xf[i * R : (i + 1) * R])
        m = {
            "x_nat": xs,
            "x_t": np.ascontiguousarray(xs.T),
            "w_t": w_t_bf,
        }
        if not rms_ones:
            m["rms"] = np.ascontiguousarray(rms_weight)
        in_maps.append(m)
    meta = dict(B=B, S=S, K=K, O=Ofeat, R=R, rms_ones=rms_ones, inv_sw127=inv_sw127)
    return in_maps, meta


def kernel(x, rms_weight, w_ternary, scale_w):
    in_maps, meta = make_in_maps(x, rms_weight, w_ternary, scale_w)
    nc = _get_nc(meta["R"], meta["K"], meta["O"], meta["inv_sw127"], meta["rms_ones"])
    res = run_bass_kernel_spmd(nc, in_maps, list(range(N_CORES)))
    outs = [np.asarray(res.results[i]["out"]) for i in range(N_CORES)]
    full = np.concatenate(outs, axis=0).reshape(meta["B"], meta["S"], meta["O"])
    return full.astype(np.float32, copy=False)


if __name__ == "__main__":
    # smoke test with random small-scale data through the full path
    rng = np.random.default_rng(0)
    B, S, D = 4, 4096, 4096
    x = rng.standard_normal((B, S, D), dtype=np.float32)
    rms_w = np.ones((D,), np.float32)
    w = (rng.integers(0, 3, size=(D, D)) - 1).astype(np.float32)
    sw = np.array([2.0], np.float32)
    out = kernel(x, rms_w, w, sw)
    print(out.shape, out.dtype)
